# revision 1
# baseline (speedup 1.0000x reference)
"""Trainium2 Bass kernel for nn_Block_mamba (SiMBA-style block: Mamba + EinFFT).

8 NeuronCores = 2 batch groups x 4-way shard of d_inner (256 ch/core).
Mamba: projections sharded, x_proj/out_proj partials all-reduced in-group;
selective scan via hardware tensor_tensor_scan along time (free dim), one
(s, d-tile) per instruction, decay dA kept in fp32, dB/h/y products bf16.
EinFFT: replicated per core; DFT-1024 as PE matmuls vs bf16 cos/sin
matrices, 4-point block FFT as vector butterflies, layer biases/relu/
softshrink fused into ScalarE activations with folded scale factors.

kernel(**inputs): full unsharded inputs -> full (2, 1024, 512) output.
"""

import numpy as np
import ml_dtypes

DIM = 512
NB = 4
BS = 128
DS = 64
DC = 4
DI = 1024
DTR = 32
BLOCKS = 2
LAM = 0.01
L = 1024

N_CORES = 8
GROUP = 4
DIL = DI // GROUP       # 256
P = 128
NDT = DIL // P          # 2
NCH = DIM // P          # 4
SGRP = 4                # scan s-values per reduce group

BF16 = ml_dtypes.bfloat16

_COMPILED = None


def _nt(s):
    return {"name": s, "tag": s}


def _build_program():
    import contextlib
    import concourse.bacc as bacc
    import concourse.mybir as mybir
    import concourse.tile as tile

    F32 = mybir.dt.float32
    BF = mybir.dt.bfloat16
    AF = mybir.ActivationFunctionType
    ALU = mybir.AluOpType
    AXX = mybir.AxisListType.X

    nc = bacc.Bacc("TRN2", target_bir_lowering=False, debug=False,
                   num_devices=N_CORES)

    _eps = nc.alloc_sbuf_tensor("const-float32-eps", [128, 1], F32)
    nc.gpsimd.memset(_eps.ap(), 1e-5)
    nc.const_aps.aps[(F32, 1e-5)] = _eps.ap()
    nc.all_engine_barrier()

    def din(name, shape, dt=F32):
        return nc.dram_tensor(name, shape, dt, kind="ExternalInput")

    xT_d = din("xT", [DIM, L])
    w_in_d = din("w_in", [DIM, DIM], BF)
    w_xp_d = din("w_xp", [DI, DTR + 2 * DS], BF)
    w_dt_d = din("w_dt", [DTR, DIL])
    w_out_d = din("w_out", [DIL, DIM], BF)
    conv_w_d = din("conv_w", [DIL, DC])
    conv_b_d = din("conv_b", [DIL, 1])
    dt_b_d = din("dt_b", [DIL, 1])
    A_d = din("A", [DIL, DS])
    Dp_d = din("Dp", [DIL, 1])
    ln_w_d = din("ln_w", [DIM, 1])
    ln_b_d = din("ln_b", [DIM, 1])
    n2_w_d = din("n2_w", [DIM, 1])
    n2_b_d = din("n2_b", [DIM, 1])
    CdF_d = din("CdF", [L, 256], BF)    # C[:, k1_loc] forward
    SdF_d = din("SdF", [L, 256], BF)
    CdI_d = din("CdI", [256, L], BF)    # C[k1_loc, :] inverse
    SdI_d = din("SdI", [256, L], BF)
    w1r_d = din("w1r", [NB, BS, BS], BF)
    w1i_d = din("w1i", [NB, BS, BS], BF)
    w1in_d = din("w1in", [NB, BS, BS], BF)
    w2r_d = din("w2r", [NB, BS, BS], BF)
    w2i_d = din("w2i", [NB, BS, BS], BF)
    w2in_d = din("w2in", [NB, BS, BS], BF)
    cb1r_d = din("cb1r", [NB, BS, 1])
    cb1i_d = din("cb1i", [NB, BS, 1])
    ssb_d = din("ssb", [NB, 4, BS, 1])
    ident_d = din("ident", [P, P])
    xO_d = nc.dram_tensor("xO", [DIM, L], F32, kind="ExternalOutput")
    xP_d = nc.dram_tensor("xP", [DIM, L], BF, kind="ExternalOutput")

    RG = [[0, 1, 2, 3], [4, 5, 6, 7]]

    with tile.TileContext(nc) as tc:
        stack = contextlib.ExitStack()
        with stack:
            wp = stack.enter_context(tc.tile_pool(name="wp", bufs=1))
            ap = stack.enter_context(tc.tile_pool(name="ap", bufs=1))
            lnp = stack.enter_context(tc.tile_pool(name="lnp", bufs=1))
            dram = stack.enter_context(
                tc.tile_pool(name="dram", bufs=1, space="DRAM"))

            x_res = [ap.tile([P, L], F32, **_nt(f"xres{k}"))
                     for k in range(NCH)]
            for k in range(NCH):
                nc.sync.dma_start(x_res[k][:], xT_d[k * P:(k + 1) * P, :])

            def wtile(src, shape, dt=F32, name=None, tag=None):
                t = wp.tile(shape, dt, name=name, tag=tag)
                nc.sync.dma_start(t[:], src)
                return t

            w_in = [wtile(w_in_d[k * P:(k + 1) * P, :], [P, DIM], BF,
                          **_nt(f"w_in{k}")) for k in range(NCH)]
            w_xp = [wtile(w_xp_d[k * P:(k + 1) * P, :], [P, DTR + 2 * DS], BF,
                          **_nt(f"w_xp{k}")) for k in range(8)]
            w_dt = wtile(w_dt_d[:], [DTR, DIL], **_nt("w_dt"))
            w_out = [wtile(w_out_d[k * P:(k + 1) * P, :], [P, DIM], BF,
                           **_nt(f"w_out{k}")) for k in range(NDT)]
            conv_w = [wtile(conv_w_d[k * P:(k + 1) * P, :], [P, DC],
                            **_nt(f"convw{k}")) for k in range(NDT)]
            conv_b = [wtile(conv_b_d[k * P:(k + 1) * P, :], [P, 1],
                            **_nt(f"convb{k}")) for k in range(NDT)]
            dt_b = [wtile(dt_b_d[k * P:(k + 1) * P, :], [P, 1],
                          **_nt(f"dtb{k}")) for k in range(NDT)]
            A_t = [wtile(A_d[k * P:(k + 1) * P, :], [P, DS], **_nt(f"A{k}"))
                   for k in range(NDT)]
            Dp = [wtile(Dp_d[k * P:(k + 1) * P, :], [P, 1], **_nt(f"Dp{k}"))
                  for k in range(NDT)]
            ln_w = [wtile(ln_w_d[k * P:(k + 1) * P, :], [P, 1],
                          **_nt(f"lnw{k}")) for k in range(NCH)]
            ln_b = [wtile(ln_b_d[k * P:(k + 1) * P, :], [P, 1],
                          **_nt(f"lnb{k}")) for k in range(NCH)]
            n2_w = [wtile(n2_w_d[k * P:(k + 1) * P, :], [P, 1],
                          **_nt(f"n2w{k}")) for k in range(NCH)]
            n2_b = [wtile(n2_b_d[k * P:(k + 1) * P, :], [P, 1],
                          **_nt(f"n2b{k}")) for k in range(NCH)]
            CdF = [wtile(CdF_d[k * P:(k + 1) * P, :], [P, 256], BF,
                         **_nt(f"CdF{k}")) for k in range(8)]
            SdF = [wtile(SdF_d[k * P:(k + 1) * P, :], [P, 256], BF,
                         **_nt(f"SdF{k}")) for k in range(8)]
            CdI = [wtile(CdI_d[c * P:(c + 1) * P, :], [P, L], BF,
                         **_nt(f"CdI{c}")) for c in range(2)]
            SdI = [wtile(SdI_d[c * P:(c + 1) * P, :], [P, L], BF,
                         **_nt(f"SdI{c}")) for c in range(2)]
            w1r = [wtile(w1r_d[b], [BS, BS], BF, **_nt(f"w1r{b}"))
                   for b in range(NB)]
            w1i = [wtile(w1i_d[b], [BS, BS], BF, **_nt(f"w1i{b}"))
                   for b in range(NB)]
            w1in = [wtile(w1in_d[b], [BS, BS], BF, **_nt(f"w1in{b}"))
                    for b in range(NB)]
            w2r = [wtile(w2r_d[b], [BS, BS], BF, **_nt(f"w2r{b}"))
                   for b in range(NB)]
            w2i = [wtile(w2i_d[b], [BS, BS], BF, **_nt(f"w2i{b}"))
                   for b in range(NB)]
            w2in = [wtile(w2in_d[b], [BS, BS], BF, **_nt(f"w2in{b}"))
                    for b in range(NB)]
            cb1r = [wtile(cb1r_d[b], [BS, 1], **_nt(f"cb1r{b}"))
                    for b in range(NB)]
            cb1i = [wtile(cb1i_d[b], [BS, 1], **_nt(f"cb1i{b}"))
                    for b in range(NB)]
            ssb = [[wtile(ssb_d[b, j], [BS, 1], **_nt(f"ssb{b}_{j}"))
                    for j in range(4)] for b in range(NB)]
            ident = wtile(ident_d[:], [P, P], **_nt("ident"))
            ident_bf = wp.tile([P, P], BF, **_nt("ident_bf"))
            nc.vector.tensor_copy(ident_bf[:], ident[:])

            ones_k1 = wp.tile([1, P], F32, **_nt("ones_k1"))
            nc.vector.memset(ones_k1[:], 1.0)
            ones_m1 = wp.tile([P, 1], F32, **_nt("ones_m1"))
            nc.vector.memset(ones_m1[:], 1.0)
            ones_bf = wp.tile([P, P], BF, **_nt("ones_bf"))
            nc.vector.memset(ones_bf[:], 1.0)

            # ----------------------------------------------------------
            def layer_norm(w_aps, b_aps, pool, out_tag):
              with tc.tile_pool(name="psln", bufs=1, space="PSUM") as ps_ln:
                  pm = ps_ln.tile([1, L], F32, **_nt("ln_mean"))
                  for k in range(NCH):
                      for h in range(2):
                          nc.tensor.matmul(
                              pm[:, h * 512:(h + 1) * 512], ones_m1[:],
                              x_res[k][:, h * 512:(h + 1) * 512],
                              start=(k == 0), stop=(k == NCH - 1))
                  psq = ps_ln.tile([1, L], F32, **_nt("ln_sq"))
                  for k in range(NCH):
                      x2 = lnp.tile([P, L], F32, **_nt("ln_x2"), bufs=2)
                      nc.scalar.activation(x2[:], x_res[k][:], AF.Square)
                      for h in range(2):
                          nc.tensor.matmul(
                              psq[:, h * 512:(h + 1) * 512], ones_m1[:],
                              x2[:, h * 512:(h + 1) * 512],
                              start=(k == 0), stop=(k == NCH - 1))
                  m = lnp.tile([1, L], F32, **_nt("ln_m"))
                  nc.vector.tensor_scalar_mul(m[:], pm[:], 1.0 / DIM)
                  ch = lnp.tile([1, L], F32, **_nt("ln_ch"))
                  nc.scalar.activation(ch[:], m[:], AF.Square)
                  nc.vector.scalar_tensor_tensor(
                      ch[:], psq[:], 1.0 / DIM, ch[:], ALU.mult, ALU.subtract)
                  inv = lnp.tile([1, L], F32, **_nt("ln_inv"))
                  nc.scalar.activation(inv[:], ch[:], AF.Abs_reciprocal_sqrt,
                                       bias=1e-5)
                  m_bc = ps_ln.tile([P, L], F32, **_nt("ln_mbc"))
                  i_bc = ps_ln.tile([P, L], F32, **_nt("ln_ibc"))
                  for h in range(2):
                      nc.tensor.matmul(m_bc[:, h * 512:(h + 1) * 512],
                                       ones_k1[:], m[:, h * 512:(h + 1) * 512],
                                       start=True, stop=True)
                      nc.tensor.matmul(i_bc[:, h * 512:(h + 1) * 512],
                                       ones_k1[:], inv[:, h * 512:(h + 1) * 512],
                                       start=True, stop=True)
                  outs = []
                  for k in range(NCH):
                      t1 = lnp.tile([P, L], F32, **_nt("ln_t1"), bufs=3)
                      nc.vector.tensor_tensor(t1[:], x_res[k][:], m_bc[:],
                                              ALU.subtract)
                      t2 = lnp.tile([P, L], F32, **_nt("ln_t2"), bufs=3)
                      nc.vector.tensor_tensor(t2[:], t1[:], i_bc[:], ALU.mult)
                      o = pool.tile([P, L], BF, **_nt(f"{out_tag}{k}"))
                      nc.vector.tensor_scalar(o[:], t2[:], w_aps[k][:],
                                              b_aps[k][:], ALU.mult, ALU.add)
                      outs.append(o)
                  return outs

            # ----------------------------------------------------------
            def mamba_block():
                with tc.tile_pool(name="mb", bufs=1) as mb:
                    xmg_in = dram.tile([DIL, L], BF, **_nt("xmgi"))
                    xmg_out = dram.tile([DI, L], BF, **_nt("xmgo"))
                    ar2_in = dram.tile([DIM, L], BF, **_nt("ar2i"))
                    ar2_out = dram.tile([DIM, L], BF, **_nt("ar2o"))

                    xm = [mb.tile([P, L], BF, **_nt(f"xm{j}"))
                          for j in range(NDT)]
                    zbf = [mb.tile([P, L], BF, **_nt(f"zbf{j}"))
                           for j in range(NDT)]
                    szs = [mb.tile([P, L], BF, **_nt(f"szs{j}"))
                           for j in range(NDT)]
                    dt = [mb.tile([P, L], F32, **_nt(f"dt{j}"))
                          for j in range(NDT)]
                    du_bf = [mb.tile([P, L], BF, **_nt(f"dubf{j}"))
                             for j in range(NDT)]
                    proj_dt = mb.tile([DTR, L], F32, **_nt("proj_dt"))
                    projbd = dram.tile([2 * DS, L], BF, **_nt("projbd"))

                    with tc.tile_pool(name="mpre", bufs=1) as mpre:
                        xn = layer_norm(ln_w, ln_b, mpre, "xn")
                        psA = tc.alloc_tile_pool(name="psA", bufs=1,
                                                 space="PSUM")
                        xm_pad = [mpre.tile([P, L + DC - 1], BF,
                                            **_nt(f"xmp{j}"))
                                  for j in range(NDT)]
                        for mt in range(4):
                            pxz = psA.tile([P, L], F32, **_nt("pxz"),
                                           bufs=2)
                            for k in range(NCH):
                                lhs = w_in[k][:, mt * P:(mt + 1) * P]
                                for h in range(2):
                                    hs = slice(h * 512, (h + 1) * 512)
                                    nc.tensor.matmul(pxz[:, hs], lhs,
                                                     xn[k][:, hs],
                                                     start=(k == 0),
                                                     stop=(k == NCH - 1))
                            j = mt % 2
                            if mt < 2:
                                nc.vector.tensor_copy(
                                    xm_pad[j][:, DC - 1:DC - 1 + L], pxz[:])
                                nc.vector.memset(xm_pad[j][:, 0:DC - 1], 0.0)
                            else:
                                nc.vector.tensor_copy(zbf[j][:], pxz[:])
                        for j in range(NDT):
                            nc.scalar.activation(szs[j][:], zbf[j][:],
                                                 AF.Silu)
                        for j in range(NDT):
                            acc = mpre.tile([P, L], BF, **_nt(f"cacc{j}"))
                            nc.vector.tensor_scalar_mul(
                                acc[:], xm_pad[j][:, 0:L], conv_w[j][:, 0:1])
                            for q in range(1, DC):
                                nc.vector.scalar_tensor_tensor(
                                    acc[:], xm_pad[j][:, q:q + L],
                                    conv_w[j][:, q:q + 1], acc[:],
                                    ALU.mult, ALU.add)
                            nc.scalar.activation(xm[j][:], acc[:], AF.Silu,
                                                 bias=conv_b[j][:])
                        for j in range(NDT):
                            nc.sync.dma_start(xmg_in[j * P:(j + 1) * P, :],
                                              xm[j][:])
                        nc.gpsimd.collective_compute(
                            "AllGather", ALU.bypass, replica_groups=RG,
                            ins=[xmg_in.opt()], outs=[xmg_out.opt()])
                        xma = [mpre.tile([P, L], BF, **_nt(f"xma{k}"))
                               for k in range(8)]
                        for k in range(8):
                            nc.sync.dma_start(xma[k][:],
                                              xmg_out[k * P:(k + 1) * P, :])
                        pp1 = psA.tile([P, L], F32, **_nt("pp1"))
                        pp2 = psA.tile([32, L], F32, **_nt("pp2"))
                        for h in range(2):
                            hs = slice(h * 512, (h + 1) * 512)
                            for k in range(8):
                                nc.tensor.matmul(pp1[:, hs], w_xp[k][:, 0:P],
                                                 xma[k][:, hs],
                                                 start=(k == 0), stop=(k == 7))
                                nc.tensor.matmul(pp2[:, hs], w_xp[k][:, P:160],
                                                 xma[k][:, hs],
                                                 start=(k == 0), stop=(k == 7))
                        nc.scalar.copy(proj_dt[:], pp1[0:DTR, :])
                        pjA = mpre.tile([P, L], BF, **_nt("pjA"))
                        nc.scalar.copy(pjA[:], pp1[:])
                        pjB = mpre.tile([32, L], BF, **_nt("pjB"))
                        nc.scalar.copy(pjB[:], pp2[:])
                        nc.sync.dma_start(projbd[0:96, :], pjA[DTR:P, :])
                        nc.sync.dma_start(projbd[96:128, :], pjB[:])
                        psA.release()

                    with tc.tile_pool(name="psD", bufs=2,
                                      space="PSUM") as psD:
                      for j in range(NDT):
                        pdt = psD.tile([P, L], F32, **_nt("pdt"))
                        for h in range(2):
                            hs = slice(h * 512, (h + 1) * 512)
                            nc.tensor.matmul(pdt[:, hs],
                                             w_dt[:, j * P:(j + 1) * P],
                                             proj_dt[:, hs],
                                             start=True, stop=True)
                        nc.scalar.activation(dt[j][:], pdt[:], AF.Exp,
                                             bias=dt_b[j][:])
                        nc.scalar.activation(dt[j][:], dt[j][:], AF.Ln,
                                             bias=1.0)
                        nc.vector.tensor_tensor(du_bf[j][:], dt[j][:],
                                                xm[j][:], ALU.mult)
                      del psD

                    with tc.tile_pool(name="msc", bufs=1) as msc:
                        yacc = [msc.tile([P, L], F32, **_nt(f"yacc{j}"))
                                for j in range(NDT)]
                        for j in range(NDT):
                            nc.vector.memset(yacc[j][:], 0.0)
                        pcur = [[None] * SGRP for _ in range(NDT)]
                        if True:
                          for sg in range(DS // SGRP):
                            for si in range(SGRP):
                                s = sg * SGRP + si
                                bB = msc.tile([P, L], BF, **_nt("bB"), bufs=3)
                                nc.sync.dma_start(
                                    bB[:],
                                    projbd[s:s + 1, :].to_broadcast((P, L)))
                                bC = msc.tile([P, L], BF, **_nt("bC"), bufs=3)
                                nc.sync.dma_start(
                                    bC[:],
                                    projbd[DS + s:DS + s + 1,
                                           :].to_broadcast((P, L)))
                                for j in range(NDT):
                                    dA = msc.tile([P, L], F32, **_nt("dA"),
                                                  bufs=3)
                                    nc.scalar.activation(
                                        dA[:], dt[j][:], AF.Exp,
                                        scale=A_t[j][:, s:s + 1])
                                    dB = msc.tile([P, L], BF, **_nt("dB"),
                                                  bufs=2)
                                    nc.vector.tensor_tensor(
                                        dB[:], du_bf[j][:], bB[:], ALU.mult)
                                    h = msc.tile([P, L], BF, **_nt("h"),
                                                 bufs=2)
                                    nc.vector.tensor_tensor_scan(
                                        h[:], dA[:], dB[:], 0.0,
                                        ALU.mult, ALU.add)
                                    p = msc.tile([P, L], BF, **_nt("pp"),
                                                 bufs=6)
                                    nc.vector.tensor_tensor(
                                        p[:], h[:], bC[:], ALU.mult)
                                    pcur[j][si] = p
                            for j in range(NDT):
                                t01 = msc.tile([P, L], BF, **_nt("t01"),
                                               bufs=2)
                                nc.vector.tensor_tensor(
                                    t01[:], pcur[j][0][:], pcur[j][1][:],
                                    ALU.add)
                                t23 = msc.tile([P, L], BF, **_nt("t23"),
                                               bufs=2)
                                nc.vector.tensor_tensor(
                                    t23[:], pcur[j][2][:], pcur[j][3][:],
                                    ALU.add)
                                tg = msc.tile([P, L], BF, **_nt("tg"),
                                              bufs=2)
                                nc.vector.tensor_tensor(
                                    tg[:], t01[:], t23[:], ALU.add)
                                nc.gpsimd.tensor_tensor(
                                    yacc[j][:], yacc[j][:], tg[:], ALU.add)
                        y2 = []
                        for j in range(NDT):
                            y1 = msc.tile([P, L], F32, **_nt("y1"), bufs=2)
                            nc.vector.scalar_tensor_tensor(
                                y1[:], xm[j][:], Dp[j][:], yacc[j][:],
                                ALU.mult, ALU.add)
                            yy = msc.tile([P, L], BF, **_nt(f"y2_{j}"))
                            nc.vector.tensor_tensor(yy[:], y1[:], szs[j][:],
                                                    ALU.mult)
                            y2.append(yy)
                        with tc.tile_pool(name="psO", bufs=2,
                                          space="PSUM") as psO:
                          for mt in range(NCH):
                            po = psO.tile([P, L], F32, **_nt("pout"))
                            for h in range(2):
                                hs = slice(h * 512, (h + 1) * 512)
                                for j in range(NDT):
                                    nc.tensor.matmul(
                                        po[:, hs],
                                        w_out[j][:, mt * P:(mt + 1) * P],
                                        y2[j][:, hs], start=(j == 0),
                                        stop=(j == NDT - 1))
                            osb = msc.tile([P, L], BF, **_nt("ar2sb"),
                                           bufs=2)
                            nc.scalar.copy(osb[:], po[:])
                            nc.sync.dma_start(
                                ar2_in[mt * P:(mt + 1) * P, :], osb[:])
                          del psO

                    nc.gpsimd.collective_compute(
                        "AllReduce", ALU.add, replica_groups=RG,
                        ins=[ar2_in.opt()], outs=[ar2_out.opt()])
                    for k in range(NCH):
                        mo = mb.tile([P, L], BF, **_nt("mo"), bufs=2)
                        nc.sync.dma_start(mo[:],
                                          ar2_out[k * P:(k + 1) * P, :])
                        nc.vector.tensor_tensor(x_res[k][:], x_res[k][:],
                                                mo[:], ALU.add)

            # ----------------------------------------------------------
            def bfly(pool, pl, tagp, W=L):
                R, I = pl[:4], pl[4:]
                t_ = {}
                for nm, (a, b, op) in {
                    "SR": (R[0], R[2], ALU.add),
                    "DR": (R[0], R[2], ALU.subtract),
                    "SR2": (R[1], R[3], ALU.add),
                    "DR2": (R[1], R[3], ALU.subtract),
                    "SI": (I[0], I[2], ALU.add),
                    "DI": (I[0], I[2], ALU.subtract),
                    "SI2": (I[1], I[3], ALU.add),
                    "DI2": (I[1], I[3], ALU.subtract),
                }.items():
                    tt = pool.tile([P, W], BF, **_nt(f"{tagp}t_{nm}"))
                    nc.vector.tensor_tensor(tt[:], a[:], b[:], op)
                    t_[nm] = tt
                spec = [("SR", "SR2", ALU.add), ("DR", "DI2", ALU.add),
                        ("SR", "SR2", ALU.subtract),
                        ("DR", "DI2", ALU.subtract),
                        ("SI", "SI2", ALU.add), ("DI", "DR2", ALU.subtract),
                        ("SI", "SI2", ALU.subtract), ("DI", "DR2", ALU.add)]
                out = []
                for i, (a, b, op) in enumerate(spec):
                    o = pool.tile([P, W], BF, **_nt(f"{tagp}o{i}"))
                    nc.vector.tensor_tensor(o[:], t_[a][:], t_[b][:], op)
                    out.append(o)
                return out[:4], out[4:]

            def einfft_block(last=False):
                KL = 256          # local k1 width
                with tc.tile_pool(name="ef", bufs=1) as ef:
                    ar3_in = dram.tile([DIM, L], BF, **_nt("ar3i"))
                    ar3_out = dram.tile([DIM, L], BF, **_nt("ar3o"))
                    Xre = [ef.tile([P, KL], BF, **_nt(f"Xre{k}"))
                           for k in range(NCH)]
                    Xim = [ef.tile([P, KL], BF, **_nt(f"Xim{k}"))
                           for k in range(NCH)]
                    with tc.tile_pool(name="efa", bufs=1) as efa:
                      xn2 = layer_norm(n2_w, n2_b, efa, "xn2")
                      xnT = [efa.tile([P, DIM], BF, **_nt(f"xnT{t}"))
                             for t in range(8)]
                      with tc.tile_pool(name="psF", bufs=1,
                                        space="PSUM") as psF:
                        for t in range(8):
                            for k in range(NCH):
                                pt = psF.tile([P, P], BF, **_nt("ptp"),
                                              bufs=2)
                                nc.tensor.transpose(
                                    pt[:], xn2[k][:, t * P:(t + 1) * P],
                                    ident_bf[:])
                                nc.vector.tensor_copy(
                                    xnT[t][:, k * P:(k + 1) * P], pt[:])
                        for k in range(NCH):
                            pre = psF.tile([P, KL], F32, **_nt("pfr"),
                                           bufs=2)
                            pim = psF.tile([P, KL], F32, **_nt("pfi"),
                                           bufs=2)
                            for t in range(8):
                                lhs = xnT[t][:, k * P:(k + 1) * P]
                                nc.tensor.matmul(pre[:], lhs, CdF[t][:],
                                                 start=(t == 0),
                                                 stop=(t == 7))
                                nc.tensor.matmul(pim[:], lhs, SdF[t][:],
                                                 start=(t == 0),
                                                 stop=(t == 7))
                            nc.vector.tensor_copy(Xre[k][:], pre[:])
                            nc.vector.tensor_scalar_mul(Xim[k][:], pim[:],
                                                        -1.0)

                    Xf_re, Xf_im = bfly(ef, Xre + Xim, "ff", KL)

                    r1 = [ef.tile([P, KL], BF, **_nt(f"r1_{b}"))
                          for b in range(NB)]
                    i1 = [ef.tile([P, KL], BF, **_nt(f"i1_{b}"))
                          for b in range(NB)]
                    with tc.tile_pool(name="psL1", bufs=2,
                                      space="PSUM") as psL1:
                      for b in range(NB):
                        pr = psL1.tile([P, KL], F32, **_nt("pl1r"))
                        nc.tensor.matmul(pr[:], w1r[b][:], Xf_re[b][:],
                                         start=True, stop=False)
                        nc.tensor.matmul(pr[:], w1in[b][:], Xf_im[b][:],
                                         start=False, stop=True)
                        nc.scalar.activation(r1[b][:], pr[:], AF.Relu,
                                             bias=cb1r[b][:])
                        pi = psL1.tile([P, KL], F32, **_nt("pl1i"))
                        nc.tensor.matmul(pi[:], w1i[b][:], Xf_re[b][:],
                                         start=True, stop=False)
                        nc.tensor.matmul(pi[:], w1r[b][:], Xf_im[b][:],
                                         start=False, stop=True)
                        nc.scalar.activation(i1[b][:], pi[:], AF.Relu,
                                             bias=cb1i[b][:])

                    zre = [None] * NB
                    zimN = [None] * NB
                    with tc.tile_pool(name="psL2", bufs=2,
                                      space="PSUM") as psL2:
                      for b in range(NB):
                        pzr = psL2.tile([P, KL], F32, **_nt("pl2r"))
                        nc.tensor.matmul(pzr[:], w2r[b][:], r1[b][:],
                                         start=True, stop=False)
                        nc.tensor.matmul(pzr[:], w2in[b][:], i1[b][:],
                                         start=False, stop=True)
                        a1 = ef.tile([P, KL], BF, **_nt("ss"), bufs=4)
                        nc.scalar.activation(a1[:], pzr[:], AF.Relu,
                                             scale=0.5, bias=ssb[b][0][:])
                        a2 = ef.tile([P, KL], BF, **_nt("ss"), bufs=4)
                        nc.scalar.activation(a2[:], pzr[:], AF.Relu,
                                             scale=-0.5, bias=ssb[b][1][:])
                        zr = ef.tile([P, KL], BF, name=f"zre{b}",
                                     tag=f"Xre{b}")
                        nc.vector.tensor_tensor(zr[:], a1[:], a2[:],
                                                ALU.subtract)
                        zre[b] = zr
                        pzi = psL2.tile([P, KL], F32, **_nt("pl2i"))
                        nc.tensor.matmul(pzi[:], w2i[b][:], r1[b][:],
                                         start=True, stop=False)
                        nc.tensor.matmul(pzi[:], w2r[b][:], i1[b][:],
                                         start=False, stop=True)
                        b1 = ef.tile([P, KL], BF, **_nt("ss"), bufs=4)
                        nc.scalar.activation(b1[:], pzi[:], AF.Relu,
                                             scale=0.5, bias=ssb[b][2][:])
                        b2 = ef.tile([P, KL], BF, **_nt("ss"), bufs=4)
                        nc.scalar.activation(b2[:], pzi[:], AF.Relu,
                                             scale=-0.5, bias=ssb[b][3][:])
                        zi = ef.tile([P, KL], BF, name=f"zimN{b}",
                                     tag=f"Xim{b}")
                        nc.vector.tensor_tensor(zi[:], b2[:], b1[:],
                                                ALU.subtract)
                        zimN[b] = zi

                    zz_re, zz_iN = bfly(ef, zre + zimN, "ff", KL)

                    with tc.tile_pool(name="psI", bufs=2,
                                      space="PSUM") as psI:
                      for b in range(NB):
                        zTr = ef.tile([P, KL], BF, **_nt("zzTr"), bufs=2)
                        zTi = ef.tile([P, KL], BF, **_nt("zzTi"), bufs=2)
                        for c in range(2):
                            pt = psI.tile([P, P], BF, **_nt("ptp2"))
                            nc.tensor.transpose(
                                pt[:], zz_re[b][:, c * P:(c + 1) * P],
                                ident_bf[:])
                            nc.vector.tensor_copy(zTr[:, c * P:(c + 1) * P],
                                                  pt[:])
                            pt2 = psI.tile([P, P], BF, **_nt("ptp3"))
                            nc.tensor.transpose(
                                pt2[:], zz_iN[b][:, c * P:(c + 1) * P],
                                ident_bf[:])
                            nc.vector.tensor_copy(zTi[:, c * P:(c + 1) * P],
                                                  pt2[:])
                        for h in range(2):
                            hs = slice(h * 512, (h + 1) * 512)
                            pout = psI.tile([P, 512], F32, **_nt("pidft"))
                            for c in range(2):
                                nc.tensor.matmul(
                                    pout[:], zTr[:, c * P:(c + 1) * P],
                                    CdI[c][:, hs], start=(c == 0),
                                    stop=False)
                                nc.tensor.matmul(
                                    pout[:], zTi[:, c * P:(c + 1) * P],
                                    SdI[c][:, hs], start=False,
                                    stop=(c == 1))
                            ob = ef.tile([P, 512], BF, **_nt("eob"), bufs=3)
                            nc.vector.tensor_copy(ob[:], pout[:])
                            if last:
                                nc.sync.dma_start(
                                    xP_d[b * P:(b + 1) * P, hs], ob[:])
                            else:
                                nc.sync.dma_start(
                                    ar3_in[b * P:(b + 1) * P, hs], ob[:])

                    if not last:
                        nc.gpsimd.collective_compute(
                            "AllReduce", ALU.add, replica_groups=RG,
                            ins=[ar3_in.opt()], outs=[ar3_out.opt()])
                        for k in range(NCH):
                            eo = ef.tile([P, L], BF, **_nt("eo"), bufs=2)
                            nc.sync.dma_start(eo[:],
                                              ar3_out[k * P:(k + 1) * P, :])
                            nc.vector.tensor_tensor(x_res[k][:], x_res[k][:],
                                                    eo[:], ALU.add)

            for blk in range(BLOCKS):
                mamba_block()
                if blk == BLOCKS - 1:
                    for k in range(NCH):
                        nc.sync.dma_start(xO_d[k * P:(k + 1) * P, :],
                                          x_res[k][:])
                einfft_block(last=(blk == BLOCKS - 1))

    nc.compile()
    return nc


# --------------------------------------------------------------------------

def _make_inmaps(inputs):
    f32 = np.float32
    x = np.asarray(inputs["x"], f32)
    in_proj_w = np.asarray(inputs["in_proj_w"], f32)
    conv_w = np.asarray(inputs["conv_w"], f32)
    conv_b = np.asarray(inputs["conv_b"], f32)
    x_proj_w = np.asarray(inputs["x_proj_w"], f32)
    dt_proj_w = np.asarray(inputs["dt_proj_w"], f32)
    dt_proj_b = np.asarray(inputs["dt_proj_b"], f32)
    A_log = np.asarray(inputs["A_log"], f32)
    Dvec = np.asarray(inputs["D"], f32)
    out_proj_w = np.asarray(inputs["out_proj_w"], f32)
    ln_w = np.asarray(inputs["ln_w"], f32)
    ln_b = np.asarray(inputs["ln_b"], f32)
    n2_w = np.asarray(inputs["norm2_w"], f32)
    n2_b = np.asarray(inputs["norm2_b"], f32)
    cw1 = np.asarray(inputs["cw1"], f32)
    cw2 = np.asarray(inputs["cw2"], f32)
    cb1 = np.asarray(inputs["cb1"], f32)
    cb2 = np.asarray(inputs["cb2"], f32)

    n = np.arange(L, dtype=np.float64)
    ang = 2.0 * np.pi * np.outer(n, n) / L
    Cdft = (np.cos(ang) / np.sqrt(L)).astype(BF16)
    Sdft = (np.sin(ang) / np.sqrt(L)).astype(BF16)
    # per-core k1 slices (einfft frequency sharding)
    CdF = [np.ascontiguousarray(Cdft[:, r * 256:(r + 1) * 256])
           for r in range(GROUP)]
    SdF = [np.ascontiguousarray(Sdft[:, r * 256:(r + 1) * 256])
           for r in range(GROUP)]
    CdI = [np.ascontiguousarray(Cdft[r * 256:(r + 1) * 256, :])
           for r in range(GROUP)]
    SdI = [np.ascontiguousarray(Sdft[r * 256:(r + 1) * 256, :])
           for r in range(GROUP)]

    ssb = np.stack([
        (cb2[0] - LAM) / 2, (-cb2[0] - LAM) / 2,
        (cb2[1] - LAM) / 2, (-cb2[1] - LAM) / 2,
    ], axis=1)[:, :, :, None]

    in_maps = []
    for core in range(N_CORES):
        g, r = divmod(core, GROUP)
        lo, hi = r * DIL, (r + 1) * DIL
        m = {
            "xT": np.ascontiguousarray(x[g].T),
            "w_in": np.ascontiguousarray(
                np.concatenate([in_proj_w[lo:hi],
                                in_proj_w[DI + lo:DI + hi]], 0).T
            ).astype(BF16),
            "w_xp": np.ascontiguousarray(x_proj_w.T).astype(BF16),
            "w_dt": np.ascontiguousarray(dt_proj_w[lo:hi].T),
            "w_out": np.ascontiguousarray(
                out_proj_w[:, lo:hi].T).astype(BF16),
            "conv_w": np.ascontiguousarray(conv_w[lo:hi, 0, :]),
            "conv_b": np.ascontiguousarray(conv_b[lo:hi][:, None]),
            "dt_b": np.ascontiguousarray(dt_proj_b[lo:hi][:, None]),
            "A": np.ascontiguousarray(-np.exp(A_log[lo:hi])),
            "Dp": np.ascontiguousarray(Dvec[lo:hi][:, None]),
            "ln_w": np.ascontiguousarray(ln_w[:, None]),
            "ln_b": np.ascontiguousarray(ln_b[:, None]),
            "n2_w": np.ascontiguousarray(n2_w[:, None]),
            "n2_b": np.ascontiguousarray(n2_b[:, None]),
            "CdF": CdF[r], "SdF": SdF[r],
            "CdI": CdI[r], "SdI": SdI[r],
            "w1r": (0.5 * cw1[0]).astype(BF16),
            "w1i": (0.5 * cw1[1]).astype(BF16),
            "w1in": (-0.5 * cw1[1]).astype(BF16),
            "w2r": cw2[0].astype(BF16),
            "w2i": cw2[1].astype(BF16),
            "w2in": (-cw2[1]).astype(BF16),
            "cb1r": np.ascontiguousarray(cb1[0][:, :, None]),
            "cb1i": np.ascontiguousarray(cb1[1][:, :, None]),
            "ssb": np.ascontiguousarray(ssb, f32),
            "ident": np.eye(P, dtype=f32),
        }
        in_maps.append(m)
    return in_maps


def kernel(**inputs):
    global _COMPILED
    from concourse.bass_utils import run_bass_kernel_spmd
    if _COMPILED is None:
        _COMPILED = _build_program()
    in_maps = _make_inmaps(inputs)
    res = run_bass_kernel_spmd(_COMPILED, in_maps,
                               core_ids=list(range(N_CORES)))
    outs = []
    for g in range(2):
        x = res.results[g * GROUP]["xO"].astype(np.float32)
        for r in range(GROUP):
            x = x + res.results[g * GROUP + r]["xP"].astype(np.float32)
        outs.append(x.T)
    return np.ascontiguousarray(np.stack(outs).astype(np.float32))



# revision 49
# speedup vs baseline: 1.4042x; 1.4042x over previous
"""Trainium2 Bass kernel for nn_Block_mamba (SiMBA-style block: Mamba + EinFFT).

Sharding: 8 cores = 4-way d_inner shard x 2-way state (s) shard; BOTH batch
elements run time-interleaved on every core so all collectives (AllReduce of
branch outputs over all 8 cores) hide behind the other batch's compute.

Mamba: in_proj/conv/silu replicated over full d_inner (kills the x_proj
AllGather); conv as 4 shifted PE matmuls with diagonal weights; selective scan
on DVE (dA fp32 from ScalarE exp, dB/h/p bf16), Sigma_s of C*h accumulated in
PSUM via identity matmuls on PE; dB/p multiplies split between DVE and Pool.
EinFFT: k1-frequency 8-way shard; DFT-1024 as PE matmuls vs bf16 cos/sin
matrices, 4-point block FFT as vector butterflies, biases/relu/softshrink
fused into ScalarE activations. Final-block einfft partials summed on host.

kernel(**inputs): full unsharded inputs -> full (2, 1024, 512) output.
"""

import numpy as np
import ml_dtypes

DIM = 512
NB = 4
BS = 128
DS = 64
DC = 4
DI = 1024
DTR = 32
BLOCKS = 2
LAM = 0.01
L = 1024
NBATCH = 2

N_CORES = 8
DSH = 4                 # d_inner shards
SSH = 2                 # s shards
DIL = DI // DSH         # 256 local channels
SL = DS // SSH          # 32 local s values
P = 128
NJT = DIL // P          # 2 local d tiles
NMT = DI // P           # 8 full-DI tiles
NCH = DIM // P          # 4 model-dim tiles
KL = L // N_CORES       # 128 local k1 bins

BF16 = ml_dtypes.bfloat16

_COMPILED = None


def _nt(s):
    return {"name": s, "tag": s}


def _build_program():
    import contextlib
    import concourse.bacc as bacc
    import concourse.mybir as mybir
    import concourse.tile as tile

    F32 = mybir.dt.float32
    BF = mybir.dt.bfloat16
    AF = mybir.ActivationFunctionType
    ALU = mybir.AluOpType

    nc = bacc.Bacc("TRN2", target_bir_lowering=False, debug=False,
                   num_devices=N_CORES)

    _eps = nc.alloc_sbuf_tensor("const-float32-eps", [128, 1], F32)
    nc.gpsimd.memset(_eps.ap(), 1e-5)
    nc.const_aps.aps[(F32, 1e-5)] = _eps.ap()
    nc.all_engine_barrier()

    def din(name, shape, dt=F32):
        return nc.dram_tensor(name, shape, dt, kind="ExternalInput")

    # host-prepped inputs (see _make_inmaps)
    xT_d = din("xT", [DIM, NBATCH * L], BF)
    lnv_d = din("lnv", [P, 4 * NCH])   # ln_w|ln_b|n2_w|n2_b, col k = tile k
    w_in_d = din("w_in", [DIM, DI + DIL], BF)      # full xm + local z columns
    cdiag_d = din("cdiag", [P, NMT * DC * P], BF)  # conv diag mats packed
    conv_b_d = din("conv_b", [P, NMT])
    # x_proj reduced to the 96 locally-needed rows: dt 0:32, B-local 32:64,
    # C-local 64:96 (s-half selected host-side); columns permuted so the
    # core's own d-channels sit in tiles 0..1 (matching w_in permutation).
    w_xp_d = din("w_xp", [P, NMT * 96], BF)        # per-k tiles packed
    w_dt_d = din("w_dt", [DTR, DIL], BF)
    mvec_d = din("mvec", [P, 2 * (SL + 2)])        # dt_b|A|Dp per j tile
    w_out_d = din("w_out", [DIL, DIM], BF)
    CdF_d = din("CdF", [P, 8 * KL], BF)            # fwd cos, per-t packed
    SdF_d = din("SdF", [P, 8 * KL], BF)            # -sin (imag folded)
    CdI_d = din("CdI", [KL, L], BF)                # DFT inv slices
    SdI_d = din("SdI", [KL, L], BF)
    w1r_d = din("w1r", [BS, NB * BS], BF)
    w1i_d = din("w1i", [BS, NB * BS], BF)
    w1in_d = din("w1in", [BS, NB * BS], BF)
    w2r_d = din("w2r", [BS, NB * BS], BF)
    w2i_d = din("w2i", [BS, NB * BS], BF)
    w2in_d = din("w2in", [BS, NB * BS], BF)
    evec_d = din("evec", [P, 2 * NB + 4 * NB])     # cb1r|cb1i|ssb packed
    ident_d = din("ident", [P, P], BF)

    xO_d = nc.dram_tensor("xO", [DIM, NBATCH * L], BF, kind="ExternalOutput")
    xP_d = nc.dram_tensor("xP", [DIM, NBATCH * L], BF, kind="ExternalOutput")

    RG = [[0, 1, 2, 3, 4, 5, 6, 7]]

    with tile.TileContext(nc) as tc:
        stack = contextlib.ExitStack()
        with stack:
            wp = stack.enter_context(tc.tile_pool(name="wp", bufs=1))
            ap = stack.enter_context(tc.tile_pool(name="ap", bufs=1))
            lnp = stack.enter_context(tc.tile_pool(name="lnp", bufs=1))
            frp = stack.enter_context(tc.tile_pool(name="frp", bufs=1))
            scp = stack.enter_context(tc.tile_pool(name="scp", bufs=1))
            efp = stack.enter_context(tc.tile_pool(name="efp", bufs=1))
            dram = stack.enter_context(
                tc.tile_pool(name="dram", bufs=1, space="DRAM"))

            # residual stream, both batches: 4 tiles [128, 2048] f32
            x_res = [ap.tile([P, NBATCH * L], BF, **_nt(f"xres{k}"))
                     for k in range(NCH)]
            for k in range(NCH):
                nc.sync.dma_start(x_res[k][:], xT_d[k * P:(k + 1) * P, :])

            def wtile(src, shape, dt=F32, name=None, tag=None):
                t = wp.tile(shape, dt, name=name, tag=tag)
                nc.sync.dma_start(t[:], src)
                return t

            # weights, packed into few DMAs, in rough first-use order
            lnv = wtile(lnv_d[:], [P, 4 * NCH], **_nt("lnv"))
            ln_w = [lnv[:, k:k + 1] for k in range(NCH)]
            ln_b = [lnv[:, NCH + k:NCH + k + 1] for k in range(NCH)]
            n2_w = [lnv[:, 2 * NCH + k:2 * NCH + k + 1] for k in range(NCH)]
            n2_b = [lnv[:, 3 * NCH + k:3 * NCH + k + 1] for k in range(NCH)]
            w_in = [wtile(w_in_d[k * P:(k + 1) * P, :], [P, DI + DIL], BF,
                          **_nt(f"w_in{k}")) for k in range(NCH)]
            cdiag_t = wtile(cdiag_d[:], [P, NMT * DC * P], BF,
                            **_nt("cdiag"))
            cdiag = [cdiag_t[:, i * P:(i + 1) * P]
                     for i in range(NMT * DC)]
            cbv = wtile(conv_b_d[:], [P, NMT], **_nt("cbv"))
            conv_b = [cbv[:, k:k + 1] for k in range(NMT)]
            w_xp_t = wtile(w_xp_d[:], [P, NMT * 96], BF, **_nt("w_xp"))
            w_xp = [w_xp_t[:, k * 96:(k + 1) * 96] for k in range(NMT)]
            w_dt = wtile(w_dt_d[:], [DTR, DIL], BF, **_nt("w_dt"))
            mvec = wtile(mvec_d[:], [P, 2 * (SL + 2)], **_nt("mvec"))
            dt_b = [mvec[:, j * (SL + 2):j * (SL + 2) + 1]
                    for j in range(NJT)]
            A_t = [mvec[:, j * (SL + 2) + 1:j * (SL + 2) + 1 + SL]
                   for j in range(NJT)]
            Dp = [mvec[:, j * (SL + 2) + 1 + SL:j * (SL + 2) + 2 + SL]
                  for j in range(NJT)]
            w_out = [wtile(w_out_d[j * P:(j + 1) * P, :], [P, DIM], BF,
                           **_nt(f"w_out{j}")) for j in range(NJT)]
            ident_bf = wtile(ident_d[:], [P, P], BF, **_nt("ident_bf"))
            CdF_t = wtile(CdF_d[:], [P, 8 * KL], BF, **_nt("CdF"))
            CdF = [CdF_t[:, t * KL:(t + 1) * KL] for t in range(8)]
            SdF_t = wtile(SdF_d[:], [P, 8 * KL], BF, **_nt("SdF"))
            SdF = [SdF_t[:, t * KL:(t + 1) * KL] for t in range(8)]
            w1r_t = wtile(w1r_d[:], [BS, NB * BS], BF, **_nt("w1r"))
            w1r = [w1r_t[:, b * BS:(b + 1) * BS] for b in range(NB)]
            w1i_t = wtile(w1i_d[:], [BS, NB * BS], BF, **_nt("w1i"))
            w1i = [w1i_t[:, b * BS:(b + 1) * BS] for b in range(NB)]
            w1in_t = wtile(w1in_d[:], [BS, NB * BS], BF, **_nt("w1in"))
            w1in = [w1in_t[:, b * BS:(b + 1) * BS] for b in range(NB)]
            w2r_t = wtile(w2r_d[:], [BS, NB * BS], BF, **_nt("w2r"))
            w2r = [w2r_t[:, b * BS:(b + 1) * BS] for b in range(NB)]
            w2i_t = wtile(w2i_d[:], [BS, NB * BS], BF, **_nt("w2i"))
            w2i = [w2i_t[:, b * BS:(b + 1) * BS] for b in range(NB)]
            w2in_t = wtile(w2in_d[:], [BS, NB * BS], BF, **_nt("w2in"))
            w2in = [w2in_t[:, b * BS:(b + 1) * BS] for b in range(NB)]
            evec = wtile(evec_d[:], [P, 2 * NB + 4 * NB], **_nt("evec"))
            cb1r = [evec[:, b:b + 1] for b in range(NB)]
            cb1i = [evec[:, NB + b:NB + b + 1] for b in range(NB)]
            ssb = [[evec[:, 2 * NB + 4 * b + j:2 * NB + 4 * b + j + 1]
                    for j in range(4)] for b in range(NB)]
            CdI = wtile(CdI_d[:], [KL, L], BF, **_nt("CdI"))
            SdI = wtile(SdI_d[:], [KL, L], BF, **_nt("SdI"))

            ones_m1 = wp.tile([P, 1], BF, **_nt("ones_m1"))
            nc.vector.memset(ones_m1[:], 1.0)
            ones_k1 = wp.tile([1, P], BF, **_nt("ones_k1"))
            nc.vector.memset(ones_k1[:], 1.0)

            # collective staging (dram)
            ar2_in = [dram.tile([DIM, L], BF, **_nt(f"ar2i{b}"))
                      for b in range(NBATCH)]
            ar2_rs = [dram.tile([DIM // 8, L], BF, **_nt(f"ar2r{b}"))
                      for b in range(NBATCH)]
            ar2_out = [dram.tile([DIM, L], BF, **_nt(f"ar2o{b}"))
                       for b in range(NBATCH)]
            ar3_in = [dram.tile([DIM, L], BF, **_nt(f"ar3i{b}"))
                      for b in range(NBATCH)]
            ar3_rs = [dram.tile([DIM // 8, L], BF, **_nt(f"ar3r{b}"))
                      for b in range(NBATCH)]
            ar3_out = [dram.tile([DIM, L], BF, **_nt(f"ar3o{b}"))
                       for b in range(NBATCH)]
            projbd = [dram.tile([2 * SL, L], BF, **_nt(f"projbd{b}"))
                      for b in range(NBATCH)]

            # ----------------------------------------------------------
            def layer_norm(b, w_aps, b_aps, pool, out_tag, sfx=""):
                """LN over channels for batch b; returns 4 bf16 [P,L] tiles."""
                cs = slice(b * L, (b + 1) * L)
                with tc.tile_pool(name="psln", bufs=1, space="PSUM") as psl:
                    pm = psl.tile([1, L], F32, **_nt("ln_pm"))
                    psq = psl.tile([1, L], F32, **_nt("ln_psq"))
                    for k in range(NCH):
                        x2 = lnp.tile([P, L], BF, **_nt("lnx2"), bufs=2)
                        nc.scalar.activation(x2[:], x_res[k][:, cs],
                                             AF.Square)
                        for h in range(2):
                            hs = slice(h * 512, (h + 1) * 512)
                            nc.tensor.matmul(pm[:, hs], ones_m1[:],
                                             x_res[k][:, cs][:, hs],
                                             start=(k == 0),
                                             stop=(k == NCH - 1))
                            nc.tensor.matmul(psq[:, hs], ones_m1[:],
                                             x2[:, hs], start=(k == 0),
                                             stop=(k == NCH - 1))
                    m_bf = lnp.tile([1, L], BF, **_nt("ln_m"))
                    nc.scalar.activation(m_bf[:], pm[:], AF.Copy,
                                         scale=1.0 / DIM)
                    m2 = lnp.tile([1, L], BF, **_nt("ln_m2"))
                    nc.scalar.activation(m2[:], m_bf[:], AF.Square)
                    var = lnp.tile([1, L], BF, **_nt("ln_var"))
                    nc.vector.scalar_tensor_tensor(
                        var[:], psq[:], 1.0 / DIM, m2[:], ALU.mult,
                        ALU.subtract)
                    inv = lnp.tile([1, L], BF, **_nt("ln_inv"))
                    nc.scalar.activation(inv[:], var[:],
                                         AF.Abs_reciprocal_sqrt, bias=1e-5)
                with tc.tile_pool(name="psbc", bufs=1, space="PSUM") as psb:
                    m_ps = psb.tile([P, L], F32, **_nt("ln_mbp"))
                    i_ps = psb.tile([P, L], F32, **_nt("ln_ibp"))
                    for h in range(2):
                        hs = slice(h * 512, (h + 1) * 512)
                        nc.tensor.matmul(m_ps[:, hs], ones_k1[:],
                                         m_bf[:, hs], start=True, stop=True)
                        nc.tensor.matmul(i_ps[:, hs], ones_k1[:],
                                         inv[:, hs], start=True, stop=True)
                    m_bc = lnp.tile([P, L], BF, **_nt("ln_mbc"))
                    nc.scalar.copy(m_bc[:], m_ps[:])
                    i_bc = lnp.tile([P, L], BF, **_nt("ln_ibc"))
                    nc.scalar.copy(i_bc[:], i_ps[:])
                outs = []
                for k in range(NCH):
                    t1 = lnp.tile([P, L], BF, **_nt("ln_t1"), bufs=2)
                    nc.vector.tensor_tensor(t1[:], x_res[k][:, cs], m_bc[:],
                                            ALU.subtract)
                    t2 = lnp.tile([P, L], BF, **_nt("ln_t2"), bufs=2)
                    nc.vector.tensor_tensor(t2[:], t1[:], i_bc[:],
                                            ALU.mult)
                    o = pool.tile([P, L], BF, **_nt(f"{out_tag}{k}"),
                                  bufs=1)
                    nc.vector.tensor_scalar(o[:], t2[:], w_aps[k][:],
                                            b_aps[k][:], ALU.mult,
                                            ALU.add)
                    outs.append(o)
                return outs

            # per-batch persistent mamba tiles
            xm = [[None] * NMT for _ in range(NBATCH)]     # full-DI silu(conv)
            szs = [[None] * NJT for _ in range(NBATCH)]    # silu(z) local
            dt_bf = [[None] * NJT for _ in range(NBATCH)]  # softplus dt bf16
            du = [[None] * NJT for _ in range(NBATCH)]     # dt*xm local bf16
            y2 = [[None] * NJT for _ in range(NBATCH)]     # gated scan out

            # ----------------------------------------------------------
            def mamba_front(b):
                """LN1 + in_proj(full DI) + conv(PE) + silu + x_proj + dt."""
                xn = layer_norm(b, ln_w, ln_b, frp, "xn_")
                with tc.tile_pool(name="psf", bufs=1, space="PSUM") as psf:
                    # in_proj (8 full-DI tiles + 2 local z tiles) with conv
                    # and the x_proj contraction fused per tile so non-local
                    # xm tiles can rotate through 2 buffers.
                    pp1 = psf.tile([96, L], F32, **_nt("pp1"))
                    for mt in range(NMT + NJT):
                        pxz = psf.tile([P, L], F32, **_nt("pxz"), bufs=2)
                        for k in range(NCH):
                            lhs = w_in[k][:, mt * P:(mt + 1) * P]
                            for h in range(2):
                                hs = slice(h * 512, (h + 1) * 512)
                                nc.tensor.matmul(pxz[:, hs], lhs,
                                                 xn[k][:, hs],
                                                 start=(k == 0),
                                                 stop=(k == NCH - 1))
                        if mt < NMT:
                            xp = frp.tile([P, L + DC - 1], BF,
                                          **_nt("xmp"), bufs=2)
                            nc.vector.memset(xp[:, 0:DC - 1], 0.0)
                            nc.vector.tensor_copy(xp[:, DC - 1:], pxz[:])
                            # conv: 4 shifted diag matmuls, then silu
                            pc = psf.tile([P, L], F32, **_nt("pconv"))
                            for h in range(2):
                                hs = slice(h * 512, (h + 1) * 512)
                                for q in range(DC):
                                    nc.tensor.matmul(
                                        pc[:, hs], cdiag[mt * DC + q][:],
                                        xp[:, h * 512 + q:
                                           h * 512 + q + 512],
                                        start=(q == 0), stop=(q == DC - 1))
                            tag = (f"xm{b}_{mt}" if mt < NJT
                                   else "xmnl")
                            xmt = frp.tile([P, L], BF, name=tag, tag=tag,
                                           bufs=(1 if mt < NJT else 2))
                            nc.scalar.activation(xmt[:], pc[:], AF.Silu,
                                                 bias=conv_b[mt][:])
                            xm[b][mt] = xmt
                            for h in range(2):
                                hs = slice(h * 512, (h + 1) * 512)
                                nc.tensor.matmul(pp1[:, hs], w_xp[mt][:],
                                                 xmt[:, hs],
                                                 start=(mt == 0),
                                                 stop=(mt == NMT - 1))
                        else:
                            j = mt - NMT
                            sz = frp.tile([P, L], BF, **_nt(f"szs{b}_{j}"))
                            nc.scalar.activation(sz[:], pxz[:], AF.Silu)
                            szs[b][j] = sz
                    # rows 0:32 dt_rank -> sbuf for dt matmul
                    proj_dt = frp.tile([DTR, L], BF, **_nt("proj_dt"))
                    nc.scalar.copy(proj_dt[:], pp1[0:DTR, :])
                    # rows 32:96 = B-local|C-local -> bf16, stage to dram
                    # (PSUM reads from partition 32/64 limited to 32-spans)
                    pjBC = frp.tile([2 * SL, L], BF, **_nt("pjBC"))
                    nc.scalar.copy(pjBC[0:SL, :], pp1[DTR:DTR + SL, :])
                    nc.scalar.copy(pjBC[SL:2 * SL, :],
                                   pp1[DTR + SL:DTR + 2 * SL, :])
                    nc.sync.dma_start(projbd[b][:], pjBC[:])

                with tc.tile_pool(name="psp", bufs=1, space="PSUM") as psp:
                    # dt chain (local channels = xm tiles 0..1 by permutation)
                    pdt = psp.tile([P, L], F32, **_nt("pdt"), bufs=2)
                    for j in range(NJT):
                        for h in range(2):
                            hs = slice(h * 512, (h + 1) * 512)
                            nc.tensor.matmul(pdt[:, hs],
                                             w_dt[:, j * P:(j + 1) * P],
                                             proj_dt[:, hs],
                                             start=True, stop=True)
                        dtj = frp.tile([P, L], BF, **_nt(f"dt{b}_{j}"))
                        nc.scalar.activation(dtj[:], pdt[:], AF.Exp,
                                             bias=dt_b[j][:])
                        nc.scalar.activation(dtj[:], dtj[:], AF.Ln, bias=1.0)
                        dt_bf[b][j] = dtj
                        duj = frp.tile([P, L], BF, **_nt(f"du{b}_{j}"))
                        nc.vector.tensor_tensor(
                            duj[:], dtj[:], xm[b][j][:], ALU.mult)
                        du[b][j] = duj

            # ----------------------------------------------------------
            def mamba_scan(b):
                """SL s-iterations; yacc in PSUM via identity matmuls."""
                with tc.tile_pool(name="psy", bufs=1, space="PSUM") as psy:
                    yacc = [psy.tile([P, L], F32, **_nt(f"yacc{j}"))
                            for j in range(NJT)]
                    for si in range(SL):
                        bB = scp.tile([P, L], BF, **_nt("bB"), bufs=3)
                        nc.sync.dma_start(
                            bB[:],
                            projbd[b][si:si + 1, :].to_broadcast((P, L)))
                        bC = scp.tile([P, L], BF, **_nt("bC"), bufs=3)
                        nc.sync.dma_start(
                            bC[:],
                            projbd[b][SL + si:SL + si + 1,
                                      :].to_broadcast((P, L)))
                        for j in range(NJT):
                            dA = scp.tile([P, L], F32, **_nt("dA"), bufs=2)
                            nc.scalar.activation(dA[:], dt_bf[b][j][:],
                                                 AF.Exp,
                                                 scale=A_t[j][:, si:si + 1])
                            dB = scp.tile([P, L], BF, **_nt("dB"), bufs=3)
                            if j == 0 and si % 2 == 0:
                                nc.gpsimd.tensor_tensor(
                                    dB[:], du[b][j][:], bB[:], ALU.mult)
                            else:
                                nc.vector.tensor_tensor(
                                    dB[:], du[b][j][:], bB[:], ALU.mult)
                            h = scp.tile([P, L], BF, **_nt("h"), bufs=3)
                            nc.vector.tensor_tensor_scan(
                                h[:], dA[:], dB[:], 0.0, ALU.mult, ALU.add)
                            p = scp.tile([P, L], BF, **_nt("p"), bufs=3)
                            if j == 0:
                                nc.gpsimd.tensor_tensor(
                                    p[:], h[:], bC[:], ALU.mult)
                            else:
                                nc.vector.tensor_tensor(
                                    p[:], h[:], bC[:], ALU.mult)
                            for hh in range(2):
                                hs = slice(hh * 512, (hh + 1) * 512)
                                nc.tensor.matmul(
                                    yacc[j][:, hs], ident_bf[:], p[:, hs],
                                    start=(si == 0), stop=(si == SL - 1))
                    # Dp is zeroed host-side on s-half-1 cores so the D*xm
                    # term is added exactly once across the AllReduce.
                    for j in range(NJT):
                        y1 = scp.tile([P, L], BF, **_nt("y1"), bufs=2)
                        nc.vector.scalar_tensor_tensor(
                            y1[:], xm[b][j][:], Dp[j][:],
                            yacc[j][:], ALU.mult, ALU.add)
                        yy = scp.tile([P, L], BF, **_nt(f"y2_{j}"))
                        nc.vector.tensor_tensor(yy[:], y1[:], szs[b][j][:],
                                                ALU.mult)
                        y2[b][j] = yy

            # ----------------------------------------------------------
            def mamba_out(b):
                with tc.tile_pool(name="pso", bufs=2, space="PSUM") as pso:
                    for mt in range(NCH):
                        po = pso.tile([P, L], F32, **_nt("pout"))
                        for h in range(2):
                            hs = slice(h * 512, (h + 1) * 512)
                            for j in range(NJT):
                                nc.tensor.matmul(
                                    po[:, hs],
                                    w_out[j][:, mt * P:(mt + 1) * P],
                                    y2[b][j][:, hs], start=(j == 0),
                                    stop=(j == NJT - 1))
                        osb = scp.tile([P, L], BF, **_nt("osb"), bufs=2)
                        nc.scalar.copy(osb[:], po[:])
                        nc.sync.dma_start(ar2_in[b][mt * P:(mt + 1) * P, :],
                                          osb[:])
                nc.gpsimd.collective_compute(
                    "ReduceScatter", ALU.add, replica_groups=RG,
                    ins=[ar2_in[b].opt()], outs=[ar2_rs[b].opt()])
                nc.gpsimd.collective_compute(
                    "AllGather", ALU.bypass, replica_groups=RG,
                    ins=[ar2_rs[b].opt()], outs=[ar2_out[b].opt()])

            def residual_add(b, src):
                cs = slice(b * L, (b + 1) * L)
                for k in range(NCH):
                    mo = scp.tile([P, L], BF, **_nt("osb"), bufs=2)
                    nc.sync.dma_start(mo[:], src[b][k * P:(k + 1) * P, :])
                    nc.gpsimd.tensor_tensor(x_res[k][:, cs], x_res[k][:, cs],
                                            mo[:], ALU.add)

            # ----------------------------------------------------------
            def bfly(pl, tagp):
                R, I = pl[:4], pl[4:]
                t_ = {}
                for nm, (a, c, op) in {
                    "SR": (R[0], R[2], ALU.add),
                    "DR": (R[0], R[2], ALU.subtract),
                    "SR2": (R[1], R[3], ALU.add),
                    "DR2": (R[1], R[3], ALU.subtract),
                    "SI": (I[0], I[2], ALU.add),
                    "DI": (I[0], I[2], ALU.subtract),
                    "SI2": (I[1], I[3], ALU.add),
                    "DI2": (I[1], I[3], ALU.subtract),
                }.items():
                    tt = efp.tile([P, KL], BF, **_nt(f"{tagp}t_{nm}"))
                    nc.vector.tensor_tensor(tt[:], a[:], c[:], op)
                    t_[nm] = tt
                spec = [("SR", "SR2", ALU.add), ("DR", "DI2", ALU.add),
                        ("SR", "SR2", ALU.subtract),
                        ("DR", "DI2", ALU.subtract),
                        ("SI", "SI2", ALU.add), ("DI", "DR2", ALU.subtract),
                        ("SI", "SI2", ALU.subtract), ("DI", "DR2", ALU.add)]
                out = []
                for i, (a, c, op) in enumerate(spec):
                    o = efp.tile([P, KL], BF, **_nt(f"{tagp}o{i}"))
                    nc.vector.tensor_tensor(o[:], t_[a][:], t_[c][:], op)
                    out.append(o)
                return out[:4], out[4:]

            def einfft(b, last, sfx=""):
                xn2 = layer_norm(b, n2_w, n2_b, frp, "xn_", sfx)
                yield
                Xre = [None] * NB
                Xim = [None] * NB
                with tc.tile_pool(name="psE", bufs=1, space="PSUM") as psE:
                    # transpose LN2 out to token-partitions (all 8 t-tiles
                    # first -- standalone transposes must not interleave
                    # with open PSUM accumulation chains), then per-block
                    # forward-DFT chains.
                    xnT = [efp.tile([P, DIM], BF, **_nt(f"xnT{t}"))
                           for t in range(8)]
                    for t in range(8):
                        for k in range(NCH):
                            pt = psE.tile([P, P], BF, **_nt("ptp"), bufs=2)
                            nc.tensor.transpose(
                                pt[:], xn2[k][:, t * P:(t + 1) * P],
                                ident_bf[:])
                            nc.vector.tensor_copy(
                                xnT[t][:, k * P:(k + 1) * P], pt[:])
                    for cb in range(NB):
                        pre = psE.tile([P, KL], F32, **_nt("pfr"), bufs=2)
                        pim = psE.tile([P, KL], F32, **_nt("pfi"), bufs=2)
                        for t in range(8):
                            lhs = xnT[t][:, cb * P:(cb + 1) * P]
                            nc.tensor.matmul(pre[:], lhs, CdF[t][:],
                                             start=(t == 0), stop=(t == 7))
                            nc.tensor.matmul(pim[:], lhs, SdF[t][:],
                                             start=(t == 0), stop=(t == 7))
                        xr = efp.tile([P, KL], BF, **_nt(f"Xre{cb}"))
                        nc.scalar.copy(xr[:], pre[:])
                        Xre[cb] = xr
                        xi = efp.tile([P, KL], BF, **_nt(f"Xim{cb}"))
                        nc.scalar.copy(xi[:], pim[:])
                        Xim[cb] = xi

                Xf_re, Xf_im = bfly(Xre + Xim, "ff")
                yield

                r1 = [None] * NB
                i1 = [None] * NB
                with tc.tile_pool(name="psL1", bufs=2, space="PSUM") as psL1:
                    for cb in range(NB):
                        pr = psL1.tile([P, KL], F32, **_nt("pl1r"))
                        nc.tensor.matmul(pr[:], w1r[cb][:], Xf_re[cb][:],
                                         start=True, stop=False)
                        nc.tensor.matmul(pr[:], w1in[cb][:], Xf_im[cb][:],
                                         start=False, stop=True)
                        r1c = efp.tile([P, KL], BF, **_nt(f"r1_{cb}"))
                        nc.scalar.activation(r1c[:], pr[:], AF.Relu,
                                             bias=cb1r[cb][:])
                        r1[cb] = r1c
                        pi = psL1.tile([P, KL], F32, **_nt("pl1i"))
                        nc.tensor.matmul(pi[:], w1i[cb][:], Xf_re[cb][:],
                                         start=True, stop=False)
                        nc.tensor.matmul(pi[:], w1r[cb][:], Xf_im[cb][:],
                                         start=False, stop=True)
                        i1c = efp.tile([P, KL], BF, **_nt(f"i1_{cb}"))
                        nc.scalar.activation(i1c[:], pi[:], AF.Relu,
                                             bias=cb1i[cb][:])
                        i1[cb] = i1c

                zre = [None] * NB
                zimN = [None] * NB
                with tc.tile_pool(name="psL2", bufs=2, space="PSUM") as psL2:
                    for cb in range(NB):
                        pzr = psL2.tile([P, KL], F32, **_nt("pl2r"))
                        nc.tensor.matmul(pzr[:], w2r[cb][:], r1[cb][:],
                                         start=True, stop=False)
                        nc.tensor.matmul(pzr[:], w2in[cb][:], i1[cb][:],
                                         start=False, stop=True)
                        a1 = efp.tile([P, KL], BF, **_nt("ssa"), bufs=4)
                        nc.scalar.activation(a1[:], pzr[:], AF.Relu,
                                             scale=0.5, bias=ssb[cb][0][:])
                        a2 = efp.tile([P, KL], BF, **_nt("ssa"), bufs=4)
                        nc.scalar.activation(a2[:], pzr[:], AF.Relu,
                                             scale=-0.5, bias=ssb[cb][1][:])
                        zr = efp.tile([P, KL], BF, **_nt(f"zre{cb}"))
                        nc.vector.tensor_tensor(zr[:], a1[:], a2[:],
                                                ALU.subtract)
                        zre[cb] = zr
                        pzi = psL2.tile([P, KL], F32, **_nt("pl2i"))
                        nc.tensor.matmul(pzi[:], w2i[cb][:], r1[cb][:],
                                         start=True, stop=False)
                        nc.tensor.matmul(pzi[:], w2r[cb][:], i1[cb][:],
                                         start=False, stop=True)
                        b1 = efp.tile([P, KL], BF, **_nt("ssa"), bufs=4)
                        nc.scalar.activation(b1[:], pzi[:], AF.Relu,
                                             scale=0.5, bias=ssb[cb][2][:])
                        b2 = efp.tile([P, KL], BF, **_nt("ssa"), bufs=4)
                        nc.scalar.activation(b2[:], pzi[:], AF.Relu,
                                             scale=-0.5, bias=ssb[cb][3][:])
                        zi = efp.tile([P, KL], BF, **_nt(f"zim{cb}"))
                        nc.vector.tensor_tensor(zi[:], b2[:], b1[:],
                                                ALU.subtract)
                        zimN[cb] = zi

                zz_re, zz_iN = bfly(zre + zimN, "ff")
                yield

                with tc.tile_pool(name="psI", bufs=2, space="PSUM") as psI:
                    for cb in range(NB):
                        zTr = efp.tile([P, KL], BF, **_nt("zzTr"), bufs=2)
                        zTi = efp.tile([P, KL], BF, **_nt("zzTi"), bufs=2)
                        pt = psI.tile([P, P], BF, **_nt("ptp2"))
                        nc.tensor.transpose(pt[:], zz_re[cb][:], ident_bf[:])
                        nc.vector.tensor_copy(zTr[:], pt[:])
                        pt2 = psI.tile([P, P], BF, **_nt("ptp3"))
                        nc.tensor.transpose(pt2[:], zz_iN[cb][:], ident_bf[:])
                        nc.vector.tensor_copy(zTi[:], pt2[:])
                        for h in range(2):
                            hs = slice(h * 512, (h + 1) * 512)
                            pout = psI.tile([P, 512], F32, **_nt("pidft"))
                            nc.tensor.matmul(pout[:], zTr[:], CdI[:, hs],
                                             start=True, stop=False)
                            nc.tensor.matmul(pout[:], zTi[:], SdI[:, hs],
                                             start=False, stop=True)
                            ob = efp.tile([P, 512], BF, **_nt("eob"), bufs=2)
                            nc.scalar.copy(ob[:], pout[:])
                            if last:
                                nc.sync.dma_start(
                                    xP_d[cb * P:(cb + 1) * P,
                                         b * L + h * 512:b * L + h * 512
                                         + 512], ob[:])
                            else:
                                nc.sync.dma_start(
                                    ar3_in[b][cb * P:(cb + 1) * P, hs],
                                    ob[:])
                if not last:
                    nc.gpsimd.collective_compute(
                        "ReduceScatter", ALU.add, replica_groups=RG,
                        ins=[ar3_in[b].opt()], outs=[ar3_rs[b].opt()])
                    nc.gpsimd.collective_compute(
                        "AllGather", ALU.bypass, replica_groups=RG,
                        ins=[ar3_rs[b].opt()], outs=[ar3_out[b].opt()])

            # ----------------------------------------------------------
            # emission order chosen so in-order engine queues overlap each
            # collective with the other batch's compute (see module doc).
            for blk in range(BLOCKS):
                last = blk == BLOCKS - 1
                for b in range(NBATCH):
                    mamba_front(b)
                for b in range(NBATCH):
                    mamba_scan(b)
                    mamba_out(b)          # issues AR2(b)
                if not last:
                    # serial einffts: eff(b1) waits AR2(b1) anyway, and
                    # interleaved emission would stall eff(b0)'s later
                    # stages behind eff(b1)'s collective wait.
                    for b in range(NBATCH):
                        residual_add(b, ar2_out)   # waits AR2(b)
                        for _ in einfft(b, last):
                            pass
                    for b in range(NBATCH):
                        residual_add(b, ar3_out)
                else:
                    # last block: both einffts are collective-free, so
                    # stage-interleave their emission to hide each other's
                    # cross-engine latency (disjoint tile tags via sfx).
                    for b in range(NBATCH):
                        residual_add(b, ar2_out)
                    for k in range(NCH):
                        nc.sync.dma_start(xO_d[k * P:(k + 1) * P, :],
                                          x_res[k][:])
                    gens = [einfft(0, True, "A"), einfft(1, True, "B")]
                    done = [False, False]
                    while not all(done):
                        for gi, g in enumerate(gens):
                            if not done[gi]:
                                try:
                                    next(g)
                                except StopIteration:
                                    done[gi] = True

    nc.compile()
    return nc


# --------------------------------------------------------------------------

def _make_inmaps(inputs):
    f32 = np.float32
    x = np.asarray(inputs["x"], f32)
    in_proj_w = np.asarray(inputs["in_proj_w"], f32)
    conv_w = np.asarray(inputs["conv_w"], f32)
    conv_b = np.asarray(inputs["conv_b"], f32)
    x_proj_w = np.asarray(inputs["x_proj_w"], f32)
    dt_proj_w = np.asarray(inputs["dt_proj_w"], f32)
    dt_proj_b = np.asarray(inputs["dt_proj_b"], f32)
    A_log = np.asarray(inputs["A_log"], f32)
    Dvec = np.asarray(inputs["D"], f32)
    out_proj_w = np.asarray(inputs["out_proj_w"], f32)
    ln_w = np.asarray(inputs["ln_w"], f32)
    ln_b = np.asarray(inputs["ln_b"], f32)
    n2_w = np.asarray(inputs["norm2_w"], f32)
    n2_b = np.asarray(inputs["norm2_b"], f32)
    cw1 = np.asarray(inputs["cw1"], f32)
    cw2 = np.asarray(inputs["cw2"], f32)
    cb1 = np.asarray(inputs["cb1"], f32)
    cb2 = np.asarray(inputs["cb2"], f32)

    n = np.arange(L, dtype=np.float64)
    ang = 2.0 * np.pi * np.outer(n, n) / L
    Cdft = (np.cos(ang) / np.sqrt(L))
    Sdft = (np.sin(ang) / np.sqrt(L))

    A_full = -np.exp(A_log)                      # (DI, DS)

    ssb = np.stack([
        (cb2[0] - LAM) / 2, (-cb2[0] - LAM) / 2,
        (cb2[1] - LAM) / 2, (-cb2[1] - LAM) / 2,
    ], axis=1)[:, :, :, None]

    xT = np.ascontiguousarray(
        np.concatenate([x[0].T, x[1].T], axis=1))     # (512, 2048)

    in_maps = []
    for core in range(N_CORES):
        ds = core % DSH
        sh = core // DSH
        lo, hi = ds * DIL, (ds + 1) * DIL
        slo, shi = sh * SL, (sh + 1) * SL
        # permutation over DI putting this core's channels in tiles 0..1
        perm = np.concatenate([np.arange(lo, hi),
                               np.arange(0, lo),
                               np.arange(hi, DI)])
        # x_proj rows: dt 0:32, local B, local C; columns permuted
        xp_sel = np.concatenate([x_proj_w[0:DTR],
                                 x_proj_w[DTR + slo:DTR + shi],
                                 x_proj_w[DTR + DS + slo:DTR + DS + shi]], 0)
        cw_p = conv_w[perm, 0, :]
        cdiag = np.zeros((P, NMT * DC * P), f32)
        for mt in range(NMT):
            for q in range(DC):
                i = mt * DC + q
                np.fill_diagonal(cdiag[:, i * P:(i + 1) * P],
                                 cw_p[mt * P:(mt + 1) * P, q])
        lnv = np.stack([ln_w.reshape(NCH, P), ln_b.reshape(NCH, P),
                        n2_w.reshape(NCH, P), n2_b.reshape(NCH, P)],
                       0).reshape(4 * NCH, P).T
        conv_bp = conv_b[perm].reshape(NMT, P).T
        w_xp_full = xp_sel[:, perm].T.astype(BF16)      # (DI, 96)
        w_xp_pk = np.concatenate(
            [w_xp_full[k * P:(k + 1) * P] for k in range(NMT)], axis=1)
        Dp_loc = Dvec[lo:hi] if sh == 0 else 0.0 * Dvec[lo:hi]
        mvec = np.zeros((P, 2 * (SL + 2)), f32)
        for j in range(NJT):
            o = j * (SL + 2)
            mvec[:, o] = dt_proj_b[lo + j * P:lo + (j + 1) * P]
            mvec[:, o + 1:o + 1 + SL] = \
                A_full[lo + j * P:lo + (j + 1) * P, slo:shi]
            mvec[:, o + 1 + SL] = Dp_loc[j * P:(j + 1) * P]
        CdF_l = Cdft[:, core * KL:(core + 1) * KL]      # (L, KL)
        SdF_l = -Sdft[:, core * KL:(core + 1) * KL]
        CdF_pk = np.concatenate(
            [CdF_l[t * P:(t + 1) * P] for t in range(8)], axis=1)
        SdF_pk = np.concatenate(
            [SdF_l[t * P:(t + 1) * P] for t in range(8)], axis=1)
        evec = np.zeros((P, 6 * NB), f32)
        for cb in range(NB):
            evec[:, cb] = cb1[0][cb]
            evec[:, NB + cb] = cb1[1][cb]
            for j in range(4):
                evec[:, 2 * NB + 4 * cb + j] = ssb[cb, j, :, 0]
        pk = lambda w: np.concatenate([w[cb] for cb in range(NB)],
                                      axis=1).astype(BF16)
        m = {
            "xT": xT.astype(BF16),
            "lnv": lnv,
            "w_in": np.ascontiguousarray(
                np.concatenate([in_proj_w[perm],
                                in_proj_w[DI + lo:DI + hi]], 0).T
            ).astype(BF16),
            "cdiag": cdiag.astype(BF16),
            "conv_b": np.ascontiguousarray(conv_bp),
            "w_xp": np.ascontiguousarray(w_xp_pk).astype(BF16),
            "w_dt": np.ascontiguousarray(dt_proj_w[lo:hi].T).astype(BF16),
            "mvec": mvec,
            "w_out": np.ascontiguousarray(
                out_proj_w[:, lo:hi].T).astype(BF16),
            "CdF": np.ascontiguousarray(CdF_pk).astype(BF16),
            "SdF": np.ascontiguousarray(SdF_pk).astype(BF16),
            "CdI": np.ascontiguousarray(
                Cdft[core * KL:(core + 1) * KL, :]).astype(BF16),
            "SdI": np.ascontiguousarray(
                Sdft[core * KL:(core + 1) * KL, :]).astype(BF16),
            "w1r": pk(0.5 * cw1[0]),
            "w1i": pk(0.5 * cw1[1]),
            "w1in": pk(-0.5 * cw1[1]),
            "w2r": pk(cw2[0]),
            "w2i": pk(cw2[1]),
            "w2in": pk(-cw2[1]),
            "evec": evec,
            "ident": np.eye(P, dtype=BF16),
        }
        in_maps.append(m)
    return in_maps


def kernel(**inputs):
    global _COMPILED
    from concourse.bass_utils import run_bass_kernel_spmd
    if _COMPILED is None:
        _COMPILED = _build_program()
    in_maps = _make_inmaps(inputs)
    res = run_bass_kernel_spmd(_COMPILED, in_maps,
                               core_ids=list(range(N_CORES)))
    xo = res.results[0]["xO"].astype(np.float32)      # (512, 2048)
    acc = xo
    for r in range(N_CORES):
        acc = acc + res.results[r]["xP"].astype(np.float32)
    out = np.stack([acc[:, 0:L].T, acc[:, L:2 * L].T])
    return np.ascontiguousarray(out.astype(np.float32))
